# revision 1
# baseline (speedup 1.0000x reference)
"""Trainium2 Bass kernel for nn_CrossAttention (B=2, N=2048, C=1024, H=16, D=64).

Strategy: sequence-parallel SPMD over 8 NeuronCores. Core i owns 512 rows of
the flattened [B*N, C] token axis (cores 0-3 = batch 0, cores 4-7 = batch 1).

Per core:
  - load x_t/x_s slices (f32->bf16 cast during DMA), transpose to [C, T] via PE
  - q^T = W_q^T x^T   (k^T likewise)  -> transposed activations [dims, tokens]
  - v   = x W_v       (natural layout [tokens, dims])
  - AllGather k^T and v across the 4 cores of the same batch
  - attention with keys on PSUM partitions: S^T = k^T-chunk^T-stationary @ q^T,
    exp on ScalarE in [128, 2048] instructions (4 heads batched), then
    O^T = [V|1]^T @ P^T accumulated over key chunks (the ones column produces
    the softmax row sums for free), normalize with reciprocal * broadcast
  - fuse: out = a^T-chunks^T @ W_fuse + b_fuse, write local 512 output rows
"""

import sys

if "/opt/trn_rl_repo" not in sys.path:
    sys.path.insert(0, "/opt/trn_rl_repo")

import numpy as np

B, N, C, H, D = 2, 2048, 1024, 16, 64
NCORES = 8
T = (B * N) // NCORES          # 512 tokens per core
NT = N                         # 2048 keys per batch
P = 128
SCALE = D ** -0.5              # 0.125
KV_K_ELEMS = C * T             # 524288  (k^T shard: [1024 dims, 512 tok])
KV_V_ELEMS = T * C             # 524288  (v shard:   [512 tok, 1024 dims])
KV_ELEMS = KV_K_ELEMS + KV_V_ELEMS
GROUPS = [[0, 1, 2, 3], [4, 5, 6, 7]]

_CACHE = {}


def _build():
    import concourse.bass as bass
    import concourse.mybir as mybir
    import concourse.tile as tile
    from concourse import bacc
    from concourse.masks import make_identity

    f32 = mybir.dt.float32
    bf16 = mybir.dt.bfloat16

    nc = bacc.Bacc("TRN2", num_devices=NCORES, debug=False, enable_asserts=False)

    x_t = nc.dram_tensor("x_t", [T, C], f32, kind="ExternalInput").ap()
    x_s = nc.dram_tensor("x_s", [T, C], f32, kind="ExternalInput").ap()
    w_q = nc.dram_tensor("W_q", [C, C], f32, kind="ExternalInput").ap()
    w_kv = nc.dram_tensor("W_kv", [C, 2 * C], f32, kind="ExternalInput").ap()
    w_f = nc.dram_tensor("W_fuse", [C, C], f32, kind="ExternalInput").ap()
    b_f = nc.dram_tensor("b_fuse", [1, C], f32, kind="ExternalInput").ap()
    out = nc.dram_tensor("out", [T, C], f32, kind="ExternalOutput").ap()

    with tile.TileContext(nc) as tc:
        import contextlib

        with contextlib.ExitStack() as stk:
            consts = stk.enter_context(tc.tile_pool(name="consts", bufs=1))
            dram = stk.enter_context(tc.tile_pool(name="dram", bufs=1, space="DRAM"))

            identity = consts.tile([P, P], bf16, name="identity")
            make_identity(nc, identity)

            bias_b = consts.tile([P, C], f32, name="bias_b")
            nc.gpsimd.dma_start(out=bias_b, in_=b_f.to_broadcast([P, C]))

            # persistent activations
            qT = [consts.tile([P, T], bf16, name=f"qT{m}") for m in range(8)]
            aT = [consts.tile([P, T], bf16, name=f"aT{c}") for c in range(8)]
            wf = [consts.tile([P, C], bf16, name=f"wf{c}") for c in range(8)]

            k_in = dram.tile([KV_K_ELEMS], bf16, name="k_in")
            v_in = dram.tile([KV_V_ELEMS], bf16, name="v_in")
            k_out = dram.tile([4 * KV_K_ELEMS], bf16, name="k_out")
            v_out = dram.tile([4 * KV_V_ELEMS], bf16, name="v_out")

            # ---- phase A1: x_s -> k^T/v projections -> allgather (critical) ----
            with tc.tile_pool(name="pa1", bufs=1) as pa, \
                 tc.tile_pool(name="pa1_ps", bufs=3, space="PSUM") as pa_ps, \
                 tc.tile_pool(name="tp1_ps", bufs=3, space="PSUM") as tp_ps:

                xs_nat = [pa.tile([P, C], bf16, name=f"xs_nat{i}") for i in range(4)]
                for i in range(4):
                    nc.gpsimd.dma_start(out=xs_nat[i], in_=x_s[i * P:(i + 1) * P, :])
                wkv = [pa.tile([P, 2 * C], bf16, name=f"wkv{c}") for c in range(8)]
                for c in range(8):
                    nc.gpsimd.dma_start(out=wkv[c], in_=w_kv[c * P:(c + 1) * P, :])

                xsT = [pa.tile([P, T], bf16, name=f"xsT{c}") for c in range(8)]
                for i in range(4):          # token tile
                    for c in range(8):      # C chunk
                        pst = tp_ps.tile([P, P], bf16, name="pst")
                        nc.tensor.transpose(
                            pst, xs_nat[i][:, c * P:(c + 1) * P], identity)
                        nc.vector.tensor_copy(
                            out=xsT[c][:, i * P:(i + 1) * P], in_=pst)

                # k^T projection -> DRAM bounce for allgather
                k_in_v = k_in.rearrange("(m p t) -> m p t", m=8, p=P, t=T)
                for m in range(8):
                    ps = pa_ps.tile([P, T], f32, name="proj_ps")
                    for c in range(8):
                        nc.tensor.matmul(ps, wkv[c][:, m * P:(m + 1) * P], xsT[c],
                                         start=(c == 0), stop=(c == 7))
                    kT_loc = pa.tile([P, T], bf16, name="kT_loc", bufs=3)
                    nc.vector.tensor_copy(out=kT_loc, in_=ps)
                    nc.sync.dma_start(out=k_in_v[m], in_=kT_loc)

                # fire the k allgather as soon as k lands; it overlaps the
                # v projection, q projection, and the attention pipeline fill
                nc.gpsimd.collective_compute(
                    "AllGather", mybir.AluOpType.bypass, replica_groups=GROUPS,
                    ins=[k_in[:].opt()], outs=[k_out[:].opt()])

                # v projection (natural layout) -> DRAM bounce
                v_in_v = v_in.rearrange("(q p c) -> q p c", q=4, p=P, c=C)
                for tt in range(4):
                    v_loc = pa.tile([P, C], bf16, name="v_loc", bufs=3)
                    for nh in range(2):
                        ps = pa_ps.tile([P, 512], f32, name="proj_ps")
                        for c in range(8):
                            nc.tensor.matmul(
                                ps,
                                xsT[c][:, tt * P:(tt + 1) * P],
                                wkv[c][:, C + nh * 512:C + (nh + 1) * 512],
                                start=(c == 0), stop=(c == 7))
                        nc.vector.tensor_copy(
                            out=v_loc[:, nh * 512:(nh + 1) * 512], in_=ps)
                    nc.sync.dma_start(out=v_in_v[tt], in_=v_loc)

            nc.gpsimd.collective_compute(
                "AllGather", mybir.AluOpType.bypass, replica_groups=GROUPS,
                ins=[v_in[:].opt()], outs=[v_out[:].opt()])

            # ---- phase A2: x_t -> q^T projection (overlaps the collectives) ----
            with tc.tile_pool(name="pa2", bufs=1) as pa, \
                 tc.tile_pool(name="pa2_ps", bufs=3, space="PSUM") as pa_ps, \
                 tc.tile_pool(name="tp2_ps", bufs=3, space="PSUM") as tp_ps:

                xt_nat = [pa.tile([P, C], bf16, name=f"xt_nat{i}") for i in range(4)]
                for i in range(4):
                    nc.gpsimd.dma_start(out=xt_nat[i], in_=x_t[i * P:(i + 1) * P, :])
                wq = [pa.tile([P, C], bf16, name=f"wq{c}") for c in range(8)]
                for c in range(8):
                    nc.gpsimd.dma_start(out=wq[c], in_=w_q[c * P:(c + 1) * P, :])

                xtT = [pa.tile([P, T], bf16, name=f"xtT{c}") for c in range(8)]
                for i in range(4):
                    for c in range(8):
                        pst = tp_ps.tile([P, P], bf16, name="pst")
                        nc.tensor.transpose(
                            pst, xt_nat[i][:, c * P:(c + 1) * P], identity)
                        nc.vector.tensor_copy(
                            out=xtT[c][:, i * P:(i + 1) * P], in_=pst)

                # q^T projection: [128 qdims, T] tiles, accumulate over C chunks
                for m in range(8):
                    ps = pa_ps.tile([P, T], f32, name="proj_ps")
                    for c in range(8):
                        nc.tensor.matmul(ps, wq[c][:, m * P:(m + 1) * P], xtT[c],
                                         start=(c == 0), stop=(c == 7))
                    nc.vector.tensor_copy(out=qT[m], in_=ps)

            # ---------------- phase B: attention ----------------
            with tc.tile_pool(name="attn", bufs=1) as attn, \
                 tc.tile_pool(name="st_ps", bufs=1, space="PSUM") as st_ps, \
                 tc.tile_pool(name="ot_ps", bufs=1, space="PSUM") as ot_ps, \
                 tc.tile_pool(name="ptp", bufs=10) as ptp, \
                 tc.tile_pool(name="sm", bufs=4) as sm:

                # gathered k^T: [128 kdims, 2048 batch tokens] x 8 tiles
                kTf = [attn.tile([P, 4 * T], bf16, name=f"kTf{m}") for m in range(8)]
                k_out_v = k_out.rearrange(
                    "(r m p t) -> m p r t", r=4, m=8, p=P, t=T)
                for m in range(8):
                    nc.sync.dma_start(
                        out=kTf[m].rearrange("p (r t) -> p r t", r=4),
                        in_=k_out_v[m])

                # gathered v staged as [V_h | 1] blocks: [128 keys, 16 heads, 65].
                # Contiguous DMA into v_full, then GpSimd restripes on-chip
                # (a strided DMA straight into vp costs ~32k tiny descriptors).
                vp = [attn.tile([P, H, D + 1], bf16, name=f"vp{kt}")
                      for kt in range(16)]
                v_out_v = v_out.rearrange(
                    "(r q p c) -> r q p c", r=4, q=4, p=P, c=C)
                with tc.tile_pool(name="vfp", bufs=4) as vfp:
                    for kt in range(16):
                        v_full = vfp.tile([P, C], bf16, name="v_full")
                        nc.sync.dma_start(out=v_full, in_=v_out_v[kt // 4, kt % 4])
                        nc.gpsimd.memset(vp[kt], 1.0)
                        nc.gpsimd.tensor_copy(
                            out=vp[kt][:, :, 0:D],
                            in_=v_full.rearrange("p (h d) -> p h d", h=H))

                # W_fuse load rides the idle DMA lanes during attention
                for c in range(8):
                    nc.gpsimd.dma_start(out=wf[c], in_=w_f[c * P:(c + 1) * P, :])

                rdram = dram.tile([H * T], mybir.dt.float32, name="rdram")
                rdram_v = rdram.rearrange("(h t) -> h t", h=H)

                def emit_st(hq, kt):
                    # scores^T for 4 heads, row-packed pairs run concurrently
                    st = st_ps.tile([P, 4, T], mybir.dt.float32, name="st")
                    for i in range(4):
                        h = 4 * hq + i
                        sub = h % 2
                        nc.tensor.matmul(
                            st[:, i, :],
                            kTf[h // 2][sub * D:(sub + 1) * D,
                                        kt * P:(kt + 1) * P],
                            qT[h // 2][sub * D:(sub + 1) * D, :],
                            start=True, stop=True,
                            tile_position=(sub * D, 0))
                    return st

                for hq in range(4):          # head quads
                    ot = [ot_ps.tile([D + 1, T], mybir.dt.float32, name="ot",
                                     tag=f"ot{i}") for i in range(4)]
                    st = emit_st(hq, 0)
                    for kt in range(16):     # key chunks of 128
                        pt = ptp.tile([P, 4, T], bf16, name="pt")
                        nc.scalar.activation(
                            pt[:], st[:],
                            mybir.ActivationFunctionType.Exp, scale=SCALE)
                        # PE program order: next chunk's scores BEFORE this
                        # chunk's P@V, so S^T(kt+1) issues as soon as the exp
                        # frees the PSUM slot and the next exp isn't stuck
                        # behind P@V(kt).
                        if kt < 15:
                            st = emit_st(hq, kt + 1)
                        elif hq < 3:
                            pass  # next quad's first S^T emitted at loop top
                        for i in range(4):
                            h = 4 * hq + i
                            nc.tensor.matmul(
                                ot[i], vp[kt][:, h, :], pt[:, i, :],
                                start=(kt == 0), stop=(kt == 15))
                    # drain PSUM: unnormalized O^T -> aT (bf16), 1/rowsum -> DRAM
                    for i in range(4):
                        h = 4 * hq + i
                        nc.vector.tensor_copy(
                            out=aT[h // 2][(h % 2) * D:(h % 2 + 1) * D, :],
                            in_=ot[i][0:D, :])
                        rc = sm.tile([1, T], mybir.dt.float32, name="rc")
                        nc.vector.reciprocal(rc, ot[i][D:D + 1, :])
                        nc.sync.dma_start(out=rdram_v[h], in_=rc)

                # normalize: aT[h] *= 1/rowsum (partition-broadcast via DRAM bounce;
                # one [64, 8, T] DMA per partition-half instead of 16 small ones)
                rb_big = attn.tile([P, 8, T], mybir.dt.float32, name="rb_big")
                for half in range(2):
                    bcast = bass.AP(
                        tensor=rdram.tensor, offset=rdram.offset + half * T,
                        ap=[[0, D], [2 * T, 8], [1, T]])
                    nc.gpsimd.dma_start(
                        out=rb_big[half * D:(half + 1) * D, :, :], in_=bcast)
                for m in range(8):
                    nc.vector.tensor_mul(out=aT[m], in0=aT[m], in1=rb_big[:, m, :])

            # ---------------- phase C: fuse projection ----------------
            with tc.tile_pool(name="fu", bufs=4) as fu, \
                 tc.tile_pool(name="fu_ps", bufs=4, space="PSUM") as fu_ps:
                for tt in range(4):
                    for nh in range(2):
                        ps = fu_ps.tile([P, 512], mybir.dt.float32, name="fps")
                        for c in range(8):
                            nc.tensor.matmul(
                                ps, aT[c][:, tt * P:(tt + 1) * P],
                                wf[c][:, nh * 512:(nh + 1) * 512],
                                start=(c == 0), stop=(c == 7))
                        ob = fu.tile([P, 512], mybir.dt.float32, name="ob")
                        nc.vector.tensor_add(
                            out=ob, in0=ps, in1=bias_b[:, nh * 512:(nh + 1) * 512])
                        nc.sync.dma_start(
                            out=out[tt * P:(tt + 1) * P, nh * 512:(nh + 1) * 512],
                            in_=ob)

    nc.compile()
    return nc


def _get_nc():
    if "nc" not in _CACHE:
        _CACHE["nc"] = _build()
    return _CACHE["nc"]


def kernel(**inputs):
    nc = _get_nc()
    from concourse import bass_utils

    x_t = np.asarray(inputs["x_t"], dtype=np.float32).reshape(B * N, C)
    x_s = np.asarray(inputs["x_s"], dtype=np.float32).reshape(B * N, C)
    w_q = np.asarray(inputs["W_q"], dtype=np.float32)
    w_kv = np.asarray(inputs["W_kv"], dtype=np.float32)
    w_f = np.asarray(inputs["W_fuse"], dtype=np.float32)
    b_f = np.asarray(inputs["b_fuse"], dtype=np.float32).reshape(1, C)

    in_maps = []
    for i in range(NCORES):
        in_maps.append({
            "x_t": x_t[i * T:(i + 1) * T],
            "x_s": x_s[i * T:(i + 1) * T],
            "W_q": w_q,
            "W_kv": w_kv,
            "W_fuse": w_f,
            "b_fuse": b_f,
        })

    res = bass_utils.run_bass_kernel_spmd(nc, in_maps, core_ids=list(range(NCORES)))
    out = np.concatenate([res.results[i]["out"] for i in range(NCORES)], axis=0)
    return out.reshape(B, N, C).astype(np.float32)


if __name__ == "__main__":
    _build()
    print("build+compile OK")



# revision 7
# speedup vs baseline: 1.1982x; 1.1982x over previous
"""Trainium2 Bass kernel for nn_CrossAttention (B=2, N=2048, C=1024, H=16, D=64).

Strategy: sequence-parallel SPMD over 8 NeuronCores. Core i owns 512 rows of
the flattened [B*N, C] token axis (cores 0-3 = batch 0, cores 4-7 = batch 1).

Schedule (per core), built so the ScalarE exp stream (~126us, the hard floor)
starts as early as possible and never stalls:
  - x_s -> xsT (PE transpose), k^T proj; K AllGather fired in 2 chunks
    (m0-3 / m4-7) so the first chunk is in flight ~20us in
  - q^T proj next (gates first S^T), then v proj; V AllGather in 2 chunks
    (tt0-1 / tt2-3) to match the tt-major P@V consumption order
  - gathered k lands directly in persistent kTf SBUF tiles (no aliasing with
    phase-A pools, so the loads wait only on the collective)
  - v lands via strided DMA straight into [V_h | 1] stationary layout
  - attention: st tiles are bf16 in PSUM, double-buffered, so S^T(kt+1)
    overlaps exp(kt); the ones column in vp gives softmax row sums for free
  - per-quad normalize (batched reciprocal + DRAM-bounce partition broadcast)
    overlaps the next quad's exp stream
  - fuse projection + bias at the end
"""

import sys

if "/opt/trn_rl_repo" not in sys.path:
    sys.path.insert(0, "/opt/trn_rl_repo")

import numpy as np

B, N, C, H, D = 2, 2048, 1024, 16, 64
NCORES = 8
T = (B * N) // NCORES          # 512 tokens per core
P = 128
SCALE = D ** -0.5              # 0.125
GROUPS = [[0, 1, 2, 3], [4, 5, 6, 7]]

# kt processing order: tt-major so P@V only needs v chunk tt after vAG chunk
# covering that tt has landed. kt identifies (r=group member, tt=token tile).
KT_ORDER = [r * 4 + tt for tt in range(4) for r in range(4)]

_CACHE = {}


def _build():
    import concourse.bass as bass
    import concourse.mybir as mybir
    import concourse.tile as tile
    from concourse import bacc
    from concourse.masks import make_identity

    f32 = mybir.dt.float32
    bf16 = mybir.dt.bfloat16

    nc = bacc.Bacc("TRN2", num_devices=NCORES, debug=False, enable_asserts=False)

    x_t = nc.dram_tensor("x_t", [T, C], f32, kind="ExternalInput").ap()
    x_s = nc.dram_tensor("x_s", [T, C], f32, kind="ExternalInput").ap()
    w_q = nc.dram_tensor("W_q", [C, C], f32, kind="ExternalInput").ap()
    w_kv = nc.dram_tensor("W_kv", [C, 2 * C], f32, kind="ExternalInput").ap()
    w_f = nc.dram_tensor("W_fuse", [C, C], f32, kind="ExternalInput").ap()
    b_f = nc.dram_tensor("b_fuse", [1, C], f32, kind="ExternalInput").ap()
    out = nc.dram_tensor("out", [T, C], f32, kind="ExternalOutput").ap()

    with tile.TileContext(nc) as tc:
        import contextlib

        with contextlib.ExitStack() as stk:
            consts = stk.enter_context(tc.tile_pool(name="consts", bufs=1))
            dram = stk.enter_context(tc.tile_pool(name="dram", bufs=1, space="DRAM"))

            identity = consts.tile([P, P], bf16, name="identity")
            make_identity(nc, identity)

            bias_b = consts.tile([P, C], f32, name="bias_b")
            nc.gpsimd.dma_start(out=bias_b, in_=b_f.to_broadcast([P, C]))

            # preload the Exp activation table so the first real exp doesn't
            # pay the 1.3us ACT_TABLE_LOAD
            dact = consts.tile([1, 2], f32, name="dact")
            nc.gpsimd.memset(dact, 0.0)
            dact2 = consts.tile([1, 2], f32, name="dact2")
            nc.scalar.activation(dact2, dact,
                                 mybir.ActivationFunctionType.Exp, scale=1.0)

            # persistent tiles (never aliased -> DMAs into them wait only on
            # their true producers)
            qT = [consts.tile([P, T], bf16, name=f"qT{m}") for m in range(8)]
            aT = [consts.tile([P, T], bf16, name=f"aT{c}") for c in range(8)]
            wf = [consts.tile([P, C], bf16, name=f"wf{c}") for c in range(8)]
            kTf = [consts.tile([P, 4, T], bf16, name=f"kTf{m}") for m in range(8)]
            vp = [consts.tile([P, H, D + 1], bf16, name=f"vp{kt}")
                  for kt in range(16)]
            for kt in range(16):
                nc.gpsimd.memset(vp[kt], 1.0)

            # DRAM bounce buffers for the collectives
            k_in1 = dram.tile([4 * P * T], bf16, name="k_in1")   # m0-3
            k_in2 = dram.tile([4 * P * T], bf16, name="k_in2")   # m4-7
            k_out1 = dram.tile([16 * P * T], bf16, name="k_out1")
            k_out2 = dram.tile([16 * P * T], bf16, name="k_out2")
            v_in1 = dram.tile([2 * P * C], bf16, name="v_in1")   # tt0-1
            v_in2 = dram.tile([2 * P * C], bf16, name="v_in2")   # tt2-3
            v_out1 = dram.tile([8 * P * C], bf16, name="v_out1")
            v_out2 = dram.tile([8 * P * C], bf16, name="v_out2")
            rdram = dram.tile([H * T], f32, name="rdram")

            k_in_v = [k_in1.rearrange("(m p t) -> m p t", m=4, p=P, t=T),
                      k_in2.rearrange("(m p t) -> m p t", m=4, p=P, t=T)]
            v_in_v = [v_in1.rearrange("(q p c) -> q p c", q=2, p=P, c=C),
                      v_in2.rearrange("(q p c) -> q p c", q=2, p=P, c=C)]
            # gathered k: [r, m, p, t] ; kTf[m] wants [p, r, t]
            k_out_v = [k_out1.rearrange("(r m p t) -> m p r t", r=4, m=4, p=P, t=T),
                       k_out2.rearrange("(r m p t) -> m p r t", r=4, m=4, p=P, t=T)]
            # gathered v: [r, q, p, c] ; vp[kt=(r,tt)] <- [p, (h d)]
            v_out_v = [v_out1.rearrange("(r q p c) -> r q p c", r=4, q=2, p=P, c=C),
                       v_out2.rearrange("(r q p c) -> r q p c", r=4, q=2, p=P, c=C)]
            rdram_v = rdram.rearrange("(h t) -> h t", h=H)

            def cc_allgather(inb, outb):
                nc.gpsimd.collective_compute(
                    "AllGather", mybir.AluOpType.bypass, replica_groups=GROUPS,
                    ins=[inb[:].opt()], outs=[outb[:].opt()])

            # ---------------- phase A: projections ----------------
            with tc.tile_pool(name="pa", bufs=1) as pa, \
                 tc.tile_pool(name="pp_ps", bufs=2, space="PSUM") as pp_ps, \
                 tc.tile_pool(name="tp_ps", bufs=3, space="PSUM") as tp_ps:

                # casting DMAs (f32->bf16) must be SWDGE: all on gpsimd, in
                # consumption order: x_s, W_k, x_t, W_q, W_v
                xs_nat = [pa.tile([P, C], bf16, name=f"xs_nat{i}") for i in range(4)]
                for i in range(4):
                    nc.gpsimd.dma_start(out=xs_nat[i], in_=x_s[i * P:(i + 1) * P, :])
                wk = [pa.tile([P, C], bf16, name=f"wk{c}") for c in range(8)]
                for c in range(8):
                    nc.gpsimd.dma_start(out=wk[c], in_=w_kv[c * P:(c + 1) * P, 0:C])
                xt_nat = [pa.tile([P, C], bf16, name=f"xt_nat{i}") for i in range(4)]
                for i in range(4):
                    nc.gpsimd.dma_start(out=xt_nat[i], in_=x_t[i * P:(i + 1) * P, :])
                wq = [pa.tile([P, C], bf16, name=f"wq{c}") for c in range(8)]
                for c in range(8):
                    nc.gpsimd.dma_start(out=wq[c], in_=w_q[c * P:(c + 1) * P, :])
                wv = [pa.tile([P, C], bf16, name=f"wv{c}") for c in range(8)]
                for c in range(8):
                    nc.gpsimd.dma_start(out=wv[c], in_=w_kv[c * P:(c + 1) * P, C:2 * C])

                def transpose_in(nat, dstT):
                    # [4 x [128, C]] -> 8 x [128 cdims, T]; copies alternate
                    # vector/gpsimd so no engine becomes the chain
                    for i in range(4):
                        for c in range(8):
                            pst = tp_ps.tile([P, P], bf16, name="pst")
                            nc.tensor.transpose(
                                pst, nat[i][:, c * P:(c + 1) * P], identity)
                            eng = nc.vector
                            eng.tensor_copy(
                                out=dstT[c][:, i * P:(i + 1) * P], in_=pst)

                xsT = [pa.tile([P, T], bf16, name=f"xsT{c}") for c in range(8)]
                transpose_in(xs_nat, xsT)

                # k^T projection, gather fired in two chunks
                for m in range(8):
                    ps = pp_ps.tile([P, T], f32, name="proj_ps")
                    for c in range(8):
                        nc.tensor.matmul(ps, wk[c][:, m * P:(m + 1) * P], xsT[c],
                                         start=(c == 0), stop=(c == 7))
                    kl = pa.tile([P, T], bf16, name="kl", bufs=3)
                    nc.vector.tensor_copy(out=kl, in_=ps)
                    nc.sync.dma_start(out=k_in_v[m // 4][m % 4], in_=kl)
                    if m == 3:
                        cc_allgather(k_in1, k_out1)
                if True:
                    cc_allgather(k_in2, k_out2)

                # q^T projection next: it gates the first S^T
                xtT = [pa.tile([P, T], bf16, name=f"xtT{c}") for c in range(8)]
                transpose_in(xt_nat, xtT)
                for m in range(8):
                    ps = pp_ps.tile([P, T], f32, name="proj_ps")
                    for c in range(8):
                        nc.tensor.matmul(ps, wq[c][:, m * P:(m + 1) * P], xtT[c],
                                         start=(c == 0), stop=(c == 7))
                    nc.vector.tensor_copy(out=qT[m], in_=ps)

                # v projection (natural layout), gather in two tt chunks
                for tt in range(4):
                    vl = pa.tile([P, C], bf16, name="vl", bufs=3)
                    for nh in range(2):
                        ps = pp_ps.tile([P, 512], f32, name="proj_ps")
                        for c in range(8):
                            nc.tensor.matmul(
                                ps,
                                xsT[c][:, tt * P:(tt + 1) * P],
                                wv[c][:, nh * 512:(nh + 1) * 512],
                                start=(c == 0), stop=(c == 7))
                        nc.vector.tensor_copy(
                            out=vl[:, nh * 512:(nh + 1) * 512], in_=ps)
                    nc.sync.dma_start(out=v_in_v[tt // 2][tt % 2], in_=vl)
                    if tt == 1:
                        cc_allgather(v_in1, v_out1)
                cc_allgather(v_in2, v_out2)

                # W_fuse rides the DMA lanes behind the other weights
                for c in range(8):
                    nc.gpsimd.dma_start(out=wf[c], in_=w_f[c * P:(c + 1) * P, :])

            # gathered k -> persistent kTf (sync queue, dep = collective only)
            for m in range(8):
                nc.sync.dma_start(out=kTf[m], in_=k_out_v[m // 4][m % 4])

            # gathered v -> vp in [V_h | 1] layout via strided-dst DMA
            # (gpsimd queue so it can't head-block the kTf loads)
            for kt in range(16):
                r, ttv = kt // 4, kt % 4
                nc.gpsimd.dma_start(
                    out=vp[kt][:, :, 0:D],
                    in_=v_out_v[ttv // 2][r, ttv % 2].rearrange(
                        "p (h d) -> p h d", h=H))

            # ---------------- phase B: attention ----------------
            # head-pair groups: pair hp = heads (2hp, 2hp+1), both served by
            # kTf[hp]/qT[hp] and drained into aT[hp]. st tiles are 2 PSUM
            # banks each, double-buffered so S^T(ki+1) overlaps exp(ki); ot
            # pairs double-buffered so P@V never waits on the drain.
            with tc.tile_pool(name="st_ps", bufs=2, space="PSUM") as st_ps, \
                 tc.tile_pool(name="ot_ps", bufs=2, space="PSUM") as ot_ps, \
                 tc.tile_pool(name="ptp", bufs=16) as ptp, \
                 tc.tile_pool(name="sm", bufs=2) as sm:

                def emit_st(hp, ki):
                    kt = KT_ORDER[ki]
                    r, tcol = kt // 4, (kt % 4) * P
                    st = st_ps.tile([P, 2, T], f32, name="st")
                    for sub in range(2):
                        nc.tensor.matmul(
                            st[:, sub, :],
                            kTf[hp][sub * D:(sub + 1) * D, r, tcol:tcol + P],
                            qT[hp][sub * D:(sub + 1) * D, :],
                            start=True, stop=True,
                            tile_position=(sub * D, 0))
                    return st

                for hp in range(8):
                    ot = [ot_ps.tile([D + 1, T], f32, name="ot", tag=f"ot{i}")
                          for i in range(2)]
                    st = emit_st(hp, 0)
                    for ki in range(16):
                        kt = KT_ORDER[ki]
                        pt = ptp.tile([P, 2, T], bf16, name="pt")
                        nc.scalar.activation(
                            pt[:], st[:],
                            mybir.ActivationFunctionType.Exp, scale=SCALE)
                        if ki < 15:
                            st = emit_st(hp, ki + 1)
                        for i in range(2):
                            h = 2 * hp + i
                            nc.tensor.matmul(
                                ot[i], vp[kt][:, h, :], pt[:, i, :],
                                start=(ki == 0), stop=(ki == 15))
                    # drain pair: unnormalized O^T -> aT[hp]; row sums staged
                    # at partition 0 then DRAM-bounced into a partition
                    # broadcast (rows 0-63 <- r[2hp], 64-127 <- r[2hp+1])
                    for i in range(2):
                        nc.vector.tensor_copy(
                            out=aT[hp][i * D:(i + 1) * D, :], in_=ot[i][0:D, :])
                        rs = sm.tile([1, T], f32, name="rs", tag=f"rs{i}")
                        nc.vector.tensor_copy(out=rs, in_=ot[i][D:D + 1, :])
                        nc.sync.dma_start(out=rdram_v[2 * hp + i], in_=rs)
                    rb = sm.tile([P, T], f32, name="rb")
                    for half in range(2):
                        bcast = bass.AP(
                            tensor=rdram.tensor,
                            offset=rdram.offset + (2 * hp + half) * T,
                            ap=[[0, D], [1, T]])
                        nc.gpsimd.dma_start(
                            out=rb[half * D:(half + 1) * D, :], in_=bcast)
                    rbi = sm.tile([P, T], f32, name="rbi")
                    nc.vector.reciprocal_approx_fast(out=rbi, in_=rb)
                    nc.vector.tensor_mul(out=aT[hp], in0=aT[hp], in1=rbi)

            # ---------------- phase C: fuse projection ----------------
            with tc.tile_pool(name="fu", bufs=4) as fu, \
                 tc.tile_pool(name="fu_ps", bufs=4, space="PSUM") as fu_ps:
                for tt in range(4):
                    for nh in range(2):
                        ps = fu_ps.tile([P, 512], f32, name="fps")
                        for c in range(8):
                            nc.tensor.matmul(
                                ps, aT[c][:, tt * P:(tt + 1) * P],
                                wf[c][:, nh * 512:(nh + 1) * 512],
                                start=(c == 0), stop=(c == 7))
                        ob = fu.tile([P, 512], f32, name="ob")
                        nc.vector.tensor_add(
                            out=ob, in0=ps, in1=bias_b[:, nh * 512:(nh + 1) * 512])
                        nc.sync.dma_start(
                            out=out[tt * P:(tt + 1) * P, nh * 512:(nh + 1) * 512],
                            in_=ob)

    nc.compile()
    return nc


def _get_nc():
    if "nc" not in _CACHE:
        _CACHE["nc"] = _build()
    return _CACHE["nc"]


def kernel(**inputs):
    nc = _get_nc()
    from concourse import bass_utils

    x_t = np.asarray(inputs["x_t"], dtype=np.float32).reshape(B * N, C)
    x_s = np.asarray(inputs["x_s"], dtype=np.float32).reshape(B * N, C)
    w_q = np.asarray(inputs["W_q"], dtype=np.float32)
    w_kv = np.asarray(inputs["W_kv"], dtype=np.float32)
    w_f = np.asarray(inputs["W_fuse"], dtype=np.float32)
    b_f = np.asarray(inputs["b_fuse"], dtype=np.float32).reshape(1, C)

    in_maps = []
    for i in range(NCORES):
        in_maps.append({
            "x_t": x_t[i * T:(i + 1) * T],
            "x_s": x_s[i * T:(i + 1) * T],
            "W_q": w_q,
            "W_kv": w_kv,
            "W_fuse": w_f,
            "b_fuse": b_f,
        })

    res = bass_utils.run_bass_kernel_spmd(nc, in_maps, core_ids=list(range(NCORES)))
    out = np.concatenate([res.results[i]["out"] for i in range(NCORES)], axis=0)
    return out.reshape(B, N, C).astype(np.float32)


if __name__ == "__main__":
    _build()
    print("build+compile OK")


# revision 8
# speedup vs baseline: 1.2959x; 1.0815x over previous
"""Trainium2 Bass kernel for nn_CrossAttention (B=2, N=2048, C=1024, H=16, D=64).

Strategy: sequence-parallel SPMD over 8 NeuronCores. Core i owns 512 rows of
the flattened [B*N, C] token axis (cores 0-3 = batch 0, cores 4-7 = batch 1).

Key design points (v3):
  - all big inputs are cast to bf16 on the HOST, so every device load is a
    fast non-casting HWDGE DMA (the SWDGE casting path delivered the first
    tile only after ~27us and delayed the whole K projection)
  - x_s/x_t are transposed by the DMA X-bar (dma_start transpose=True)
    straight from DRAM -> no PE transposes, no drain copies
  - PE order: k proj -> fire K AllGather in 2 chunks -> v proj -> fire V
    AllGather in 2 chunks (tt-major) -> q proj -> attention -> fuse.
    Collectives are issued from the otherwise-empty gpsimd queue.
  - gathered k lands in persistent kTf SBUF tiles (loads wait only on the
    collective), gathered v lands via strided DMA directly in the
    [V_h | 1] stationary layout (ones column -> softmax row sums for free)
  - attention in head pairs: st [128,2,512] f32 PSUM double-buffered so
    S^T(ki+1) overlaps exp(ki); the two S^T matmuls of a pair run
    concurrently in different PE row-tiles (tile_position)
  - deep pt pool so the exp stream can run ~32 iterations ahead of P@V
    while the V gather is still in flight
  - per-pair normalize: row sums staged at partition 0, DRAM-bounce
    partition broadcast, reciprocal_approx_fast, one multiply
"""

import sys

if "/opt/trn_rl_repo" not in sys.path:
    sys.path.insert(0, "/opt/trn_rl_repo")

import numpy as np

B, N, C, H, D = 2, 2048, 1024, 16, 64
NCORES = 8
T = (B * N) // NCORES          # 512 tokens per core
P = 128
SCALE = D ** -0.5              # 0.125
GROUPS = [[0, 1, 2, 3], [4, 5, 6, 7]]

# kt processing order: tt-major so P@V only needs v chunk tt after the vAG
# chunk covering that tt has landed. kt identifies (r=group member, tt).
KT_ORDER = [r * 4 + tt for tt in range(4) for r in range(4)]

_CACHE = {}


def _build():
    import concourse.bass as bass
    import concourse.mybir as mybir
    import concourse.tile as tile
    from concourse import bacc

    f32 = mybir.dt.float32
    bf16 = mybir.dt.bfloat16

    nc = bacc.Bacc("TRN2", num_devices=NCORES, debug=False, enable_asserts=False)

    x_t = nc.dram_tensor("x_t", [T, C], bf16, kind="ExternalInput").ap()
    x_s = nc.dram_tensor("x_s", [T, C], bf16, kind="ExternalInput").ap()
    w_q = nc.dram_tensor("W_q", [C, C], bf16, kind="ExternalInput").ap()
    w_kv = nc.dram_tensor("W_kv", [C, 2 * C], bf16, kind="ExternalInput").ap()
    w_f = nc.dram_tensor("W_fuse", [C, C], bf16, kind="ExternalInput").ap()
    b_f = nc.dram_tensor("b_fuse", [1, C], f32, kind="ExternalInput").ap()
    out = nc.dram_tensor("out", [T, C], f32, kind="ExternalOutput").ap()

    with tile.TileContext(nc) as tc:
        import contextlib

        with contextlib.ExitStack() as stk:
            consts = stk.enter_context(tc.tile_pool(name="consts", bufs=1))
            dram = stk.enter_context(tc.tile_pool(name="dram", bufs=1, space="DRAM"))

            bias_b = consts.tile([P, C], f32, name="bias_b")
            nc.gpsimd.dma_start(out=bias_b, in_=b_f.to_broadcast([P, C]))

            # preload the Exp activation table (saves 1.3us at first exp)
            dact = consts.tile([1, 2], f32, name="dact")
            nc.gpsimd.memset(dact, 0.0)
            dact2 = consts.tile([1, 2], f32, name="dact2")
            nc.scalar.activation(dact2, dact,
                                 mybir.ActivationFunctionType.Exp, scale=1.0)

            qT = [consts.tile([P, T], bf16, name=f"qT{m}") for m in range(8)]
            aT = [consts.tile([P, T], bf16, name=f"aT{c}") for c in range(8)]
            wf = [consts.tile([P, C], bf16, name=f"wf{c}") for c in range(8)]
            kTf = [consts.tile([P, 4, T], bf16, name=f"kTf{m}") for m in range(8)]
            vp = [consts.tile([P, H, D + 1], bf16, name=f"vp{kt}")
                  for kt in range(16)]
            for kt in range(16):
                nc.gpsimd.memset(vp[kt], 1.0)

            # DRAM bounce buffers for the collectives
            k_in1 = dram.tile([4 * P * T], bf16, name="k_in1")   # m0-3
            k_in2 = dram.tile([4 * P * T], bf16, name="k_in2")   # m4-7
            k_out1 = dram.tile([16 * P * T], bf16, name="k_out1")
            k_out2 = dram.tile([16 * P * T], bf16, name="k_out2")
            v_in1 = dram.tile([2 * P * C], bf16, name="v_in1")   # tt0-1
            v_in2 = dram.tile([2 * P * C], bf16, name="v_in2")   # tt2-3
            v_out1 = dram.tile([8 * P * C], bf16, name="v_out1")
            v_out2 = dram.tile([8 * P * C], bf16, name="v_out2")
            rdram = dram.tile([H * T], f32, name="rdram")

            k_in_v = [k_in1.rearrange("(m p t) -> m p t", m=4, p=P, t=T),
                      k_in2.rearrange("(m p t) -> m p t", m=4, p=P, t=T)]
            v_in_v = [v_in1.rearrange("(q p c) -> q p c", q=2, p=P, c=C),
                      v_in2.rearrange("(q p c) -> q p c", q=2, p=P, c=C)]
            k_out_v = [k_out1.rearrange("(r m p t) -> m p r t", r=4, m=4, p=P, t=T),
                       k_out2.rearrange("(r m p t) -> m p r t", r=4, m=4, p=P, t=T)]
            v_out_v = [v_out1.rearrange("(r q p c) -> r q p c", r=4, q=2, p=P, c=C),
                       v_out2.rearrange("(r q p c) -> r q p c", r=4, q=2, p=P, c=C)]
            rdram_v = rdram.rearrange("(h t) -> h t", h=H)

            def cc_allgather(inb, outb):
                nc.gpsimd.collective_compute(
                    "AllGather", mybir.AluOpType.bypass, replica_groups=GROUPS,
                    ins=[inb[:].opt()], outs=[outb[:].opt()])

            # ---------------- phase A: projections ----------------
            with tc.tile_pool(name="pa", bufs=1) as pa, \
                 tc.tile_pool(name="pp_ps", bufs=2, space="PSUM") as pp_ps:

                # x^T via DMA X-bar transpose straight from DRAM (sync queue)
                xsT = [pa.tile([P, T], bf16, name=f"xsT{c}") for c in range(8)]
                for c in range(8):
                    nc.sync.dma_start(out=xsT[c],
                                      in_=x_s[:, c * P:(c + 1) * P],
                                      transpose=True)
                wk = [pa.tile([P, C], bf16, name=f"wk{c}") for c in range(8)]
                for c in range(8):
                    nc.sync.dma_start(out=wk[c], in_=w_kv[c * P:(c + 1) * P, 0:C])
                wv = [pa.tile([P, C], bf16, name=f"wv{c}") for c in range(8)]
                for c in range(8):
                    nc.sync.dma_start(out=wv[c], in_=w_kv[c * P:(c + 1) * P, C:2 * C])
                xtT = [pa.tile([P, T], bf16, name=f"xtT{c}") for c in range(8)]
                for c in range(8):
                    nc.sync.dma_start(out=xtT[c],
                                      in_=x_t[:, c * P:(c + 1) * P],
                                      transpose=True)
                wq = [pa.tile([P, C], bf16, name=f"wq{c}") for c in range(8)]
                for c in range(8):
                    nc.sync.dma_start(out=wq[c], in_=w_q[c * P:(c + 1) * P, :])
                for c in range(8):
                    nc.sync.dma_start(out=wf[c], in_=w_f[c * P:(c + 1) * P, :])

                # k^T projection; K AllGather fired in two chunks.
                # k_in writes go on the (empty) gpsimd queue so the collective
                # fires the moment the data lands.
                for m in range(8):
                    ps = pp_ps.tile([P, T], f32, name="proj_ps")
                    for c in range(8):
                        nc.tensor.matmul(ps, wk[c][:, m * P:(m + 1) * P], xsT[c],
                                         start=(c == 0), stop=(c == 7))
                    kl = pa.tile([P, T], bf16, name="kl", bufs=3)
                    nc.vector.tensor_copy(out=kl, in_=ps)
                    nc.gpsimd.dma_start(out=k_in_v[m // 4][m % 4], in_=kl)
                    if m == 3:
                        cc_allgather(k_in1, k_out1)
                cc_allgather(k_in2, k_out2)

                # v projection (natural layout), gather in two tt chunks
                for tt in range(4):
                    vl = pa.tile([P, C], bf16, name="vl", bufs=3)
                    for nh in range(2):
                        ps = pp_ps.tile([P, 512], f32, name="proj_ps")
                        for c in range(8):
                            nc.tensor.matmul(
                                ps,
                                xsT[c][:, tt * P:(tt + 1) * P],
                                wv[c][:, nh * 512:(nh + 1) * 512],
                                start=(c == 0), stop=(c == 7))
                        nc.vector.tensor_copy(
                            out=vl[:, nh * 512:(nh + 1) * 512], in_=ps)
                    nc.gpsimd.dma_start(out=v_in_v[tt // 2][tt % 2], in_=vl)
                    if tt == 1:
                        cc_allgather(v_in1, v_out1)
                cc_allgather(v_in2, v_out2)

                # q^T projection
                for m in range(8):
                    ps = pp_ps.tile([P, T], f32, name="proj_ps")
                    for c in range(8):
                        nc.tensor.matmul(ps, wq[c][:, m * P:(m + 1) * P], xtT[c],
                                         start=(c == 0), stop=(c == 7))
                    nc.vector.tensor_copy(out=qT[m], in_=ps)

            # gathered k -> persistent kTf (sync queue, dep = collective only)
            for m in range(8):
                nc.sync.dma_start(out=kTf[m], in_=k_out_v[m // 4][m % 4])

            # gathered v -> vp in [V_h | 1] layout via strided-dst DMA, in
            # tt-major order to match KT_ORDER consumption (gpsimd queue)
            for kt in KT_ORDER:
                r, ttv = kt // 4, kt % 4
                nc.gpsimd.dma_start(
                    out=vp[kt][:, :, 0:D],
                    in_=v_out_v[ttv // 2][r, ttv % 2].rearrange(
                        "p (h d) -> p h d", h=H))

            # ---------------- phase B: attention ----------------
            with tc.tile_pool(name="st_ps", bufs=2, space="PSUM") as st_ps, \
                 tc.tile_pool(name="ot_ps", bufs=2, space="PSUM") as ot_ps, \
                 tc.tile_pool(name="ptp", bufs=28) as ptp, \
                 tc.tile_pool(name="sm", bufs=2) as sm:

                def emit_st(hp, ki):
                    kt = KT_ORDER[ki]
                    r, tcol = kt // 4, (kt % 4) * P
                    st = st_ps.tile([P, 2, T], f32, name="st")
                    for sub in range(2):
                        nc.tensor.matmul(
                            st[:, sub, :],
                            kTf[hp][sub * D:(sub + 1) * D, r, tcol:tcol + P],
                            qT[hp][sub * D:(sub + 1) * D, :],
                            start=True, stop=True,
                            tile_position=(sub * D, 0))
                    return st

                for hp in range(8):
                    ot = [ot_ps.tile([D + 1, T], f32, name="ot", tag=f"ot{i}")
                          for i in range(2)]
                    st = emit_st(hp, 0)
                    for ki in range(16):
                        kt = KT_ORDER[ki]
                        pt = ptp.tile([P, 2, T], bf16, name="pt")
                        nc.scalar.activation(
                            pt[:], st[:],
                            mybir.ActivationFunctionType.Exp, scale=SCALE)
                        if ki < 15:
                            st = emit_st(hp, ki + 1)
                        for i in range(2):
                            h = 2 * hp + i
                            nc.tensor.matmul(
                                ot[i], vp[kt][:, h, :], pt[:, i, :],
                                start=(ki == 0), stop=(ki == 15))
                    # drain pair: O^T -> aT[hp]; row sums staged at partition
                    # 0, DRAM-bounced into a partition broadcast, then one
                    # fast reciprocal + multiply normalizes the pair
                    for i in range(2):
                        nc.vector.tensor_copy(
                            out=aT[hp][i * D:(i + 1) * D, :], in_=ot[i][0:D, :])
                        rs = sm.tile([1, T], f32, name="rs", tag=f"rs{i}")
                        nc.vector.tensor_copy(out=rs, in_=ot[i][D:D + 1, :])
                        nc.sync.dma_start(out=rdram_v[2 * hp + i], in_=rs)
                    rb = sm.tile([P, T], f32, name="rb")
                    for half in range(2):
                        bcast = bass.AP(
                            tensor=rdram.tensor,
                            offset=rdram.offset + (2 * hp + half) * T,
                            ap=[[0, D], [1, T]])
                        nc.gpsimd.dma_start(
                            out=rb[half * D:(half + 1) * D, :], in_=bcast)
                    rbi = sm.tile([P, T], f32, name="rbi")
                    nc.vector.reciprocal_approx_fast(out=rbi, in_=rb)
                    nc.vector.tensor_mul(out=aT[hp], in0=aT[hp], in1=rbi)

            # ---------------- phase C: fuse projection ----------------
            with tc.tile_pool(name="fu", bufs=4) as fu, \
                 tc.tile_pool(name="fu_ps", bufs=4, space="PSUM") as fu_ps:
                for tt in range(4):
                    for nh in range(2):
                        ps = fu_ps.tile([P, 512], f32, name="fps")
                        for c in range(8):
                            nc.tensor.matmul(
                                ps, aT[c][:, tt * P:(tt + 1) * P],
                                wf[c][:, nh * 512:(nh + 1) * 512],
                                start=(c == 0), stop=(c == 7))
                        ob = fu.tile([P, 512], f32, name="ob")
                        nc.vector.tensor_add(
                            out=ob, in0=ps, in1=bias_b[:, nh * 512:(nh + 1) * 512])
                        nc.sync.dma_start(
                            out=out[tt * P:(tt + 1) * P, nh * 512:(nh + 1) * 512],
                            in_=ob)

    nc.compile()
    return nc


def _get_nc():
    if "nc" not in _CACHE:
        _CACHE["nc"] = _build()
    return _CACHE["nc"]


def make_in_maps(inputs):
    """Shard + host-cast the full inputs into per-core input maps."""
    import ml_dtypes

    bf16 = ml_dtypes.bfloat16
    x_t = np.asarray(inputs["x_t"]).reshape(B * N, C).astype(bf16)
    x_s = np.asarray(inputs["x_s"]).reshape(B * N, C).astype(bf16)
    w_q = np.asarray(inputs["W_q"]).astype(bf16)
    w_kv = np.asarray(inputs["W_kv"]).astype(bf16)
    w_f = np.asarray(inputs["W_fuse"]).astype(bf16)
    b_f = np.asarray(inputs["b_fuse"]).astype(np.float32).reshape(1, C)

    in_maps = []
    for i in range(NCORES):
        in_maps.append({
            "x_t": x_t[i * T:(i + 1) * T],
            "x_s": x_s[i * T:(i + 1) * T],
            "W_q": w_q,
            "W_kv": w_kv,
            "W_fuse": w_f,
            "b_fuse": b_f,
        })
    return in_maps


def kernel(**inputs):
    nc = _get_nc()
    from concourse import bass_utils

    in_maps = make_in_maps(inputs)
    res = bass_utils.run_bass_kernel_spmd(nc, in_maps, core_ids=list(range(NCORES)))
    out = np.concatenate([res.results[i]["out"] for i in range(NCORES)], axis=0)
    return out.reshape(B, N, C).astype(np.float32)


if __name__ == "__main__":
    _build()
    print("build+compile OK")


# revision 11
# speedup vs baseline: 1.3255x; 1.0229x over previous
"""Trainium2 Bass kernel for nn_CrossAttention (B=2, N=2048, C=1024, H=16, D=64).

Strategy: sequence-parallel SPMD over 8 NeuronCores. Core i owns 512 rows of
the flattened [B*N, C] token axis (cores 0-3 = batch 0, cores 4-7 = batch 1).

Key design points (v3):
  - all big inputs are cast to bf16 on the HOST, so every device load is a
    fast non-casting HWDGE DMA (the SWDGE casting path delivered the first
    tile only after ~27us and delayed the whole K projection)
  - x_s/x_t are transposed by the DMA X-bar (dma_start transpose=True)
    straight from DRAM -> no PE transposes, no drain copies
  - PE order: k proj -> fire K AllGather in 2 chunks -> v proj -> fire V
    AllGather in 2 chunks (tt-major) -> q proj -> attention -> fuse.
    Collectives are issued from the otherwise-empty gpsimd queue.
  - gathered k lands in persistent kTf SBUF tiles (loads wait only on the
    collective), gathered v lands via strided DMA directly in the
    [V_h | 1] stationary layout (ones column -> softmax row sums for free)
  - attention in head pairs: st [128,2,512] f32 PSUM double-buffered so
    S^T(ki+1) overlaps exp(ki); the two S^T matmuls of a pair run
    concurrently in different PE row-tiles (tile_position)
  - deep pt pool so the exp stream can run ~32 iterations ahead of P@V
    while the V gather is still in flight
  - per-pair normalize: row sums staged at partition 0, DRAM-bounce
    partition broadcast, reciprocal_approx_fast, one multiply
"""

import sys

if "/opt/trn_rl_repo" not in sys.path:
    sys.path.insert(0, "/opt/trn_rl_repo")

import numpy as np

B, N, C, H, D = 2, 2048, 1024, 16, 64
NCORES = 8
T = (B * N) // NCORES          # 512 tokens per core
P = 128
SCALE = D ** -0.5              # 0.125
GROUPS = [[0, 1, 2, 3], [4, 5, 6, 7]]

# kt processing order: tt-major so P@V only needs v chunk tt after the vAG
# chunk covering that tt has landed. kt identifies (r=group member, tt).
KT_ORDER = [r * 4 + tt for tt in range(4) for r in range(4)]

_CACHE = {}


def _build():
    import concourse.bass as bass
    import concourse.mybir as mybir
    import concourse.tile as tile
    from concourse import bacc

    f32 = mybir.dt.float32
    bf16 = mybir.dt.bfloat16

    nc = bacc.Bacc("TRN2", num_devices=NCORES, debug=False, enable_asserts=False)

    x_t = nc.dram_tensor("x_t", [T, C], bf16, kind="ExternalInput").ap()
    x_s = nc.dram_tensor("x_s", [T, C], bf16, kind="ExternalInput").ap()
    w_q = nc.dram_tensor("W_q", [C, C], bf16, kind="ExternalInput").ap()
    w_kv = nc.dram_tensor("W_kv", [C, 2 * C], bf16, kind="ExternalInput").ap()
    w_f = nc.dram_tensor("W_fuse", [C, C], bf16, kind="ExternalInput").ap()
    b_f = nc.dram_tensor("b_fuse", [1, C], f32, kind="ExternalInput").ap()
    out = nc.dram_tensor("out", [T, C], f32, kind="ExternalOutput").ap()

    with tile.TileContext(nc) as tc:
        import contextlib

        with contextlib.ExitStack() as stk:
            consts = stk.enter_context(tc.tile_pool(name="consts", bufs=1))
            dram = stk.enter_context(tc.tile_pool(name="dram", bufs=1, space="DRAM"))

            # preload the Exp activation table (saves 1.3us at first exp)
            dact = consts.tile([1, 2], f32, name="dact")
            nc.vector.memset(dact, 0.0)
            dact2 = consts.tile([1, 2], f32, name="dact2")
            nc.scalar.activation(dact2, dact,
                                 mybir.ActivationFunctionType.Exp, scale=1.0)

            bias_b = consts.tile([P, C], f32, name="bias_b")
            qT = [consts.tile([P, T], bf16, name=f"qT{m}") for m in range(8)]
            aT = [consts.tile([P, T], bf16, name=f"aT{c}") for c in range(8)]
            wf = [consts.tile([P, C], bf16, name=f"wf{c}") for c in range(8)]
            kTf = [consts.tile([P, 4, T], bf16, name=f"kTf{m}") for m in range(8)]
            vp = [consts.tile([P, H, D + 1], bf16, name=f"vp{kt}")
                  for kt in range(16)]
            for kt in range(16):
                nc.vector.memset(vp[kt], 1.0)

            # DRAM bounce buffers for the collectives
            k_in1 = dram.tile([4 * P * T], bf16, name="k_in1")   # m0-3
            k_in2 = dram.tile([4 * P * T], bf16, name="k_in2")   # m4-7
            k_out1 = dram.tile([16 * P * T], bf16, name="k_out1")
            k_out2 = dram.tile([16 * P * T], bf16, name="k_out2")
            v_in1 = dram.tile([2 * P * C], bf16, name="v_in1")   # tt0-1
            v_in2 = dram.tile([2 * P * C], bf16, name="v_in2")   # tt2-3
            v_out1 = dram.tile([8 * P * C], bf16, name="v_out1")
            v_out2 = dram.tile([8 * P * C], bf16, name="v_out2")
            rdram = dram.tile([H * T], f32, name="rdram")

            k_in_v = [k_in1.rearrange("(m p t) -> m p t", m=4, p=P, t=T),
                      k_in2.rearrange("(m p t) -> m p t", m=4, p=P, t=T)]
            v_in_v = [v_in1.rearrange("(q p c) -> q p c", q=2, p=P, c=C),
                      v_in2.rearrange("(q p c) -> q p c", q=2, p=P, c=C)]
            k_out_v = [k_out1.rearrange("(r m p t) -> m p r t", r=4, m=4, p=P, t=T),
                       k_out2.rearrange("(r m p t) -> m p r t", r=4, m=4, p=P, t=T)]
            v_out_v = [v_out1.rearrange("(r q p c) -> r q p c", r=4, q=2, p=P, c=C),
                       v_out2.rearrange("(r q p c) -> r q p c", r=4, q=2, p=P, c=C)]
            rdram_v = rdram.rearrange("(h t) -> h t", h=H)

            def cc_allgather(inb, outb):
                nc.gpsimd.collective_compute(
                    "AllGather", mybir.AluOpType.bypass, replica_groups=GROUPS,
                    ins=[inb[:].opt()], outs=[outb[:].opt()])

            # ---------------- phase A: projections ----------------
            with tc.tile_pool(name="pa", bufs=1) as pa, \
                 tc.tile_pool(name="kp_ps", bufs=1, space="PSUM") as kp_ps, \
                 tc.tile_pool(name="pp_ps", bufs=2, space="PSUM") as pp_ps:

                # x^T via DMA X-bar transpose straight from DRAM. x_s on the
                # sync queue, x_t on the scalar queue (both HWDGE) so the
                # 1.3us-per-transpose issue cost runs in parallel.
                xsT = [pa.tile([P, T], bf16, name=f"xsT{c}") for c in range(8)]
                for c in range(8):
                    nc.sync.dma_start(out=xsT[c],
                                      in_=x_s[:, c * P:(c + 1) * P],
                                      transpose=True)
                xtT = [pa.tile([P, T], bf16, name=f"xtT{c}") for c in range(8)]
                for c in range(8):
                    nc.scalar.dma_start(out=xtT[c],
                                        in_=x_t[:, c * P:(c + 1) * P],
                                        transpose=True)
                # weights on the gpsimd queue, in consumption order
                wk = [pa.tile([P, C], bf16, name=f"wk{c}") for c in range(8)]
                for c in range(8):
                    nc.gpsimd.dma_start(out=wk[c], in_=w_kv[c * P:(c + 1) * P, 0:C])
                wq = [pa.tile([P, C], bf16, name=f"wq{c}") for c in range(8)]
                for c in range(8):
                    nc.gpsimd.dma_start(out=wq[c], in_=w_q[c * P:(c + 1) * P, :])
                wv = [pa.tile([P, C], bf16, name=f"wv{c}") for c in range(8)]
                for c in range(8):
                    nc.gpsimd.dma_start(out=wv[c], in_=w_kv[c * P:(c + 1) * P, C:2 * C])

                # k^T projection, c-outer over two m-groups so matmuls start
                # as soon as the first xsT/wk chunks land; K AllGather fired
                # per group from the gpsimd queue
                for mg in range(2):
                    pss = [kp_ps.tile([P, T], f32, name="kps", tag=f"kps{j}")
                           for j in range(4)]
                    for c in range(8):
                        for j in range(4):
                            m = 4 * mg + j
                            nc.tensor.matmul(
                                pss[j], wk[c][:, m * P:(m + 1) * P], xsT[c],
                                start=(c == 0), stop=(c == 7))
                    for j in range(4):
                        kl = pa.tile([P, T], bf16, name="kl", bufs=4)
                        nc.vector.tensor_copy(out=kl, in_=pss[j])
                        nc.gpsimd.dma_start(out=k_in_v[mg][j], in_=kl)
                    cc_allgather([k_in1, k_in2][mg], [k_out1, k_out2][mg])

                # q^T projection (gates the first S^T, so before v)
                for m in range(8):
                    ps = pp_ps.tile([P, T], f32, name="proj_ps")
                    for c in range(8):
                        nc.tensor.matmul(ps, wq[c][:, m * P:(m + 1) * P], xtT[c],
                                         start=(c == 0), stop=(c == 7))
                    nc.vector.tensor_copy(out=qT[m], in_=ps)

                # v projection (natural layout), gather in two tt chunks
                for tt in range(4):
                    vl = pa.tile([P, C], bf16, name="vl", bufs=3)
                    for nh in range(2):
                        ps = pp_ps.tile([P, 512], f32, name="proj_ps")
                        for c in range(8):
                            nc.tensor.matmul(
                                ps,
                                xsT[c][:, tt * P:(tt + 1) * P],
                                wv[c][:, nh * 512:(nh + 1) * 512],
                                start=(c == 0), stop=(c == 7))
                        nc.vector.tensor_copy(
                            out=vl[:, nh * 512:(nh + 1) * 512], in_=ps)
                    nc.gpsimd.dma_start(out=v_in_v[tt // 2][tt % 2], in_=vl)
                    if tt == 1:
                        cc_allgather(v_in1, v_out1)
                cc_allgather(v_in2, v_out2)

                # late loads: W_fuse + bias ride behind the collectives
                for c in range(8):
                    nc.gpsimd.dma_start(out=wf[c], in_=w_f[c * P:(c + 1) * P, :])
                nc.gpsimd.dma_start(out=bias_b, in_=b_f.to_broadcast([P, C]))

            # gathered k -> persistent kTf (sync queue, dep = collective only)
            for m in range(8):
                nc.sync.dma_start(out=kTf[m], in_=k_out_v[m // 4][m % 4])

            # gathered v -> vp in [V_h | 1] layout via strided-dst DMA, in
            # tt-major order to match KT_ORDER consumption (gpsimd queue)
            for kt in KT_ORDER:
                r, ttv = kt // 4, kt % 4
                nc.gpsimd.dma_start(
                    out=vp[kt][:, :, 0:D],
                    in_=v_out_v[ttv // 2][r, ttv % 2].rearrange(
                        "p (h d) -> p h d", h=H))

            # ---------------- phase B: attention ----------------
            with tc.tile_pool(name="st_ps", bufs=2, space="PSUM") as st_ps, \
                 tc.tile_pool(name="ot_ps", bufs=2, space="PSUM") as ot_ps, \
                 tc.tile_pool(name="ptp", bufs=40) as ptp, \
                 tc.tile_pool(name="sm", bufs=2) as sm:

                def emit_st(hp, ki):
                    kt = KT_ORDER[ki]
                    r, tcol = kt // 4, (kt % 4) * P
                    st = st_ps.tile([P, 2, T], f32, name="st")
                    for sub in range(2):
                        nc.tensor.matmul(
                            st[:, sub, :],
                            kTf[hp][sub * D:(sub + 1) * D, r, tcol:tcol + P],
                            qT[hp][sub * D:(sub + 1) * D, :],
                            start=True, stop=True,
                            tile_position=(sub * D, 0))
                    return st

                for hp in range(8):
                    ot = [ot_ps.tile([D + 1, T], f32, name="ot", tag=f"ot{i}")
                          for i in range(2)]
                    st = emit_st(hp, 0)
                    for ki in range(16):
                        kt = KT_ORDER[ki]
                        pt = ptp.tile([P, 2, T], bf16, name="pt")
                        nc.scalar.activation(
                            pt[:], st[:],
                            mybir.ActivationFunctionType.Exp, scale=SCALE)
                        if ki < 15:
                            st = emit_st(hp, ki + 1)
                        for i in range(2):
                            h = 2 * hp + i
                            nc.tensor.matmul(
                                ot[i], vp[kt][:, h, :], pt[:, i, :],
                                start=(ki == 0), stop=(ki == 15))
                    # drain pair: O^T -> aT[hp]; row sums staged at partition
                    # 0, DRAM-bounced into a partition broadcast, then one
                    # fast reciprocal + multiply normalizes the pair
                    for i in range(2):
                        nc.vector.tensor_copy(
                            out=aT[hp][i * D:(i + 1) * D, :], in_=ot[i][0:D, :])
                        rs = sm.tile([1, T], f32, name="rs", tag=f"rs{i}")
                        nc.vector.tensor_copy(out=rs, in_=ot[i][D:D + 1, :])
                        nc.sync.dma_start(out=rdram_v[2 * hp + i], in_=rs)
                    rb = sm.tile([P, T], f32, name="rb")
                    for half in range(2):
                        bcast = bass.AP(
                            tensor=rdram.tensor,
                            offset=rdram.offset + (2 * hp + half) * T,
                            ap=[[0, D], [1, T]])
                        nc.gpsimd.dma_start(
                            out=rb[half * D:(half + 1) * D, :], in_=bcast)
                    rbi = sm.tile([P, T], f32, name="rbi")
                    nc.vector.reciprocal_approx_fast(out=rbi, in_=rb)
                    nc.vector.tensor_mul(out=aT[hp], in0=aT[hp], in1=rbi)

            # ---------------- phase C: fuse projection ----------------
            with tc.tile_pool(name="fu", bufs=4) as fu, \
                 tc.tile_pool(name="fu_ps", bufs=4, space="PSUM") as fu_ps:
                for tt in range(4):
                    for nh in range(2):
                        ps = fu_ps.tile([P, 512], f32, name="fps")
                        for c in range(8):
                            nc.tensor.matmul(
                                ps, aT[c][:, tt * P:(tt + 1) * P],
                                wf[c][:, nh * 512:(nh + 1) * 512],
                                start=(c == 0), stop=(c == 7))
                        ob = fu.tile([P, 512], f32, name="ob")
                        nc.vector.tensor_add(
                            out=ob, in0=ps, in1=bias_b[:, nh * 512:(nh + 1) * 512])
                        nc.sync.dma_start(
                            out=out[tt * P:(tt + 1) * P, nh * 512:(nh + 1) * 512],
                            in_=ob)

    nc.compile()
    return nc


def _get_nc():
    if "nc" not in _CACHE:
        _CACHE["nc"] = _build()
    return _CACHE["nc"]


def make_in_maps(inputs):
    """Shard + host-cast the full inputs into per-core input maps."""
    import ml_dtypes

    bf16 = ml_dtypes.bfloat16
    x_t = np.asarray(inputs["x_t"]).reshape(B * N, C).astype(bf16)
    x_s = np.asarray(inputs["x_s"]).reshape(B * N, C).astype(bf16)
    w_q = np.asarray(inputs["W_q"]).astype(bf16)
    w_kv = np.asarray(inputs["W_kv"]).astype(bf16)
    w_f = np.asarray(inputs["W_fuse"]).astype(bf16)
    b_f = np.asarray(inputs["b_fuse"]).astype(np.float32).reshape(1, C)

    in_maps = []
    for i in range(NCORES):
        in_maps.append({
            "x_t": x_t[i * T:(i + 1) * T],
            "x_s": x_s[i * T:(i + 1) * T],
            "W_q": w_q,
            "W_kv": w_kv,
            "W_fuse": w_f,
            "b_fuse": b_f,
        })
    return in_maps


def kernel(**inputs):
    nc = _get_nc()
    from concourse import bass_utils

    in_maps = make_in_maps(inputs)
    res = bass_utils.run_bass_kernel_spmd(nc, in_maps, core_ids=list(range(NCORES)))
    out = np.concatenate([res.results[i]["out"] for i in range(NCORES)], axis=0)
    return out.reshape(B, N, C).astype(np.float32)


if __name__ == "__main__":
    _build()
    print("build+compile OK")
